# revision 19
# baseline (speedup 1.0000x reference)
# Block-circulant linear kernel for Trainium2 (Bass/Tile), 8-core SPMD.
#
# y[b, 16m+p] = sum_{n,q} blocks[(m-n)%512, p, q] * x[b, 16n+q]
#
# CRT split (exact): z^512-1 = (z^256-1)(z^256+1). With
#   Xc[n] = x[n] + x[n+256],  Xs[n] = x[n] - x[n+256]   (block index n)
#   Bc[s] = B[s] + B[s+256],  Bs[s] = B[s] - B[s+256]
# the problem becomes a 256-cyclic conv (Bc*Xc) and a 256-negacyclic conv
# (Bs*Xs); y[m] = (Yc[m]+Ys[m])/2, y[m+256] = (Yc[m]-Ys[m])/2. The fold /
# unfold are O(input) host-side adds; the device still performs the full
# contraction at half the MACs and ~0.45x the DMA bytes of the direct form.
#
# Sharding (2x2 per subproblem): each core computes a PARTIAL result for
# 128 output block-rows (m-half) over 128 contraction blocks (n-half);
# the host adds the two n-half partials. This doubles the PSUM tile count
# to 16 (full [128,512] PSUM bank, N=512 matmuls) while shrinking the
# BIGQ window to exactly L block-columns:
#   BIGQ[(ni,q), u*16+p] = sgn * Bh[(m0-n0 + u-ni - 128) % 256, p, q]
# All (m_tile t, n_chunk c) pairs with the same diagonal d = t - c share
# one stationary 128x128 tile: 31 accumulating matmuls per core.
#
# The xt layout is reversed (c' = 15 - c) and the psum t axis flipped
# (t' = 15 - t) so both streams are consumed in DMA arrival order.
import numpy as np

B = 32
NB = 512           # number of 16x16 blocks in the original problem
L = 256            # folded subproblem length (cyclic / negacyclic)
NCORES = 8
MH = 128           # output block-rows per core (m-half)
NH = 128           # contraction blocks per core (n-half)
NCH_C = NH // 8    # 16 n-chunks of 128 contraction rows
NT = MH // 8       # 16 psum tiles of 8 block-rows x 32 batch
ND = NCH_C + NT - 1  # 31 diagonal offsets d in [-15, 15]
W = 8 * ND + 8     # 256 BIGQ window width (in u units of 16 columns)
OFF = 8 * NCH_C    # window index offset: j = (m-n) + OFF
XT_COLS = NCH_C * B    # 512
OUT_COLS = NT * B      # 512

DTYPE = "float16"
NWARM = 1   # logical fp32 warm-up matmuls (2 instrs each, clock-boost nudge)
EARLY_COPY = True

_cached = {}
_last_results = None  # BassKernelResults of the most recent run (for profiling)


def _np_dtype(name):
    if name == "bfloat16":
        import ml_dtypes

        return ml_dtypes.bfloat16
    if name == "float16":
        return np.float16
    return np.float32


def _split_dt(dt_name):
    if ":" in dt_name:
        wt, mv = dt_name.split(":")
        return wt, mv
    return dt_name, dt_name


def _build_program(dt_name):
    import concourse.bacc as bacc
    import concourse.mybir as mybir
    import concourse.tile as tile

    wt_name, mv_name = _split_dt(dt_name)
    wdt = getattr(mybir.dt, wt_name)
    mdt = getattr(mybir.dt, mv_name)
    f32 = mybir.dt.float32

    nc = bacc.Bacc("TRN2", target_bir_lowering=False, debug=False, num_devices=NCORES)
    xt_d = nc.declare_dram_parameter("xt", [128, XT_COLS], mdt, isOutput=False)
    bq_d = nc.declare_dram_parameter("bigq", [128, W * 16], wdt, isOutput=False)
    out_d = nc.declare_dram_parameter("out", [128, OUT_COLS], mdt, isOutput=True)

    # bigq chunks (cols). Chunk-completion semaphores fire ~1.2us after the
    # last byte (HBM write-receipt), so the PE trails DMA by that latency.
    bq_cuts = [0, 1024, 2048, 3072, 4096]

    with tile.TileContext(nc) as tc:
        with (
            tc.tile_pool(name="data", bufs=1) as data_pool,
            tc.tile_pool(name="psum", bufs=1, space="PSUM") as psum_pool,
        ):
            xt = data_pool.tile([128, XT_COLS], mdt)
            bq = data_pool.tile([128, W * 16], wdt)
            out_sb = data_pool.tile([128, OUT_COLS], mdt)
            warm_sb = data_pool.tile([128, 256], f32)
            acc = psum_pool.tile([128, OUT_COLS], f32)
            warm_ps = psum_pool.tile([128, 256], f32)

            # two HWDGE rings (sync=SP, scalar=ACT); consumption order.
            nc.scalar.dma_start(bq[:, bq_cuts[0]:bq_cuts[1]],
                                bq_d[:, bq_cuts[0]:bq_cuts[1]])
            nc.sync.dma_start(xt[:], xt_d[:])
            nc.sync.dma_start(bq[:, bq_cuts[1]:bq_cuts[2]],
                              bq_d[:, bq_cuts[1]:bq_cuts[2]])
            nc.scalar.dma_start(bq[:, bq_cuts[2]:bq_cuts[3]],
                                bq_d[:, bq_cuts[2]:bq_cuts[3]])
            nc.sync.dma_start(bq[:, bq_cuts[3]:bq_cuts[4]],
                              bq_d[:, bq_cuts[3]:bq_cuts[4]])

            # PE warm-up nudge while DMA streams in (the HAM clock gate is
            # usually already warm from the preceding NEFF execution).
            nwarm = NWARM if mv_name in ("float16", "bfloat16") else 0
            if nwarm:
                nc.gpsimd.memset(warm_sb[:], 0.0)
            for wi in range(nwarm):
                nc.tensor.matmul(
                    warm_ps[:], warm_sb[:, 0:128], warm_sb[:],
                    start=(wi == 0), stop=(wi == nwarm - 1),
                )

            # d = t - c diagonal; stationary tile = BIGQ cols [16u0, 16u0+128)
            for i in range(ND):
                d = i - (NCH_C - 1)
                u0 = 8 * i + 8
                t_lo = max(0, d)
                t_hi = min(NT - 1, NCH_C - 1 + d)
                nt = t_hi - t_lo + 1
                tp_lo = NT - 1 - t_hi            # flipped psum tile index
                cp_lo = NCH_C - 1 + d - t_hi     # reversed xt chunk index
                nc.tensor.matmul(
                    acc[:, 32 * tp_lo: 32 * (tp_lo + nt)],
                    bq[:, 16 * u0: 16 * u0 + 128],
                    xt[:, 32 * cp_lo: 32 * (cp_lo + nt)],
                    start=(i == 0),   # clears the whole PSUM bank
                    stop=(i == ND - 1),
                )
                if EARLY_COPY and i == ND - 8:
                    # psum tiles t=0..8 (cols 256:512) got their last
                    # accumulation by i = NCH_C-1+t <= ND-8; cast them out
                    # while the remaining diagonals accumulate cols 0:256.
                    nc.scalar.copy(out_sb[:, 256:512], acc[:, 256:512])
                    nc.scalar.dma_start(out_d[:, 256:512], out_sb[:, 256:512])

            if EARLY_COPY:
                nc.vector.tensor_copy(out_sb[:, 0:256], acc[:, 0:256])
                nc.sync.dma_start(out_d[:, 0:256], out_sb[:, 0:256])
            else:
                nc.vector.tensor_copy(out_sb[:], acc[:])
                nc.sync.dma_start(out_d[:], out_sb[:])
    nc.compile()
    return nc


def _get_program(dt_name):
    key = (dt_name, NWARM, EARLY_COPY)
    if key not in _cached:
        _cached[key] = _build_program(dt_name)
    return _cached[key]


def _xt_layout(xh):
    """[32, 16*NH] n-half -> [128, XT_COLS]: xt[(ni*16+q), c'*32+b] with
    c' = NCH_C-1-c reversed chunk order."""
    xt = (
        xh.T.reshape(NCH_C, 128, B).transpose(1, 0, 2)[:, ::-1, :]
        .reshape(128, XT_COLS)
    )
    return np.ascontiguousarray(xt)


def _prep_inputs(x, blocks, dt_name):
    """Host-side fold + layout prep (numpy ops on the small inputs)."""
    x = np.ascontiguousarray(np.asarray(x), dtype=np.float32)
    blocks = np.ascontiguousarray(np.asarray(blocks), dtype=np.float32)
    wt_name, mv_name = _split_dt(dt_name)
    np_w, np_m = _np_dtype(wt_name), _np_dtype(mv_name)

    xc = x[:, : 16 * L] + x[:, 16 * L:]
    xs = x[:, : 16 * L] - x[:, 16 * L:]
    bc = blocks[:L] + blocks[L:]
    bs = blocks[:L] - blocks[L:]

    # xt per n-half: [cyc/neg][n-half]
    xts = [[_xt_layout(xh[:, 16 * NH * nh: 16 * NH * (nh + 1)]).astype(np_m)
            for nh in range(2)] for xh in (xc, xs)]

    u = np.arange(W)
    ni = np.arange(8)
    base = u[None, :] - ni[:, None] - OFF            # [8, W]
    in_maps = []
    for k in range(NCORES):
        neg = k >= 4
        mh, nh = ((k % 4) >> 1) & 1, k % 2
        s = (mh - nh) * NH + base                    # true (m-n) difference
        idx = s % L
        bh = bs if neg else bc
        bigq = bh[idx]                               # [8, W, p, q]
        if neg:
            sgn = (1.0 - 2.0 * ((s // L) % 2)).astype(np.float32)
            bigq = bigq * sgn[:, :, None, None]
        bigq = bigq.transpose(0, 3, 1, 2).reshape(128, W * 16)  # [(ni,q),(u,p)]
        in_maps.append({
            "xt": xts[1 if neg else 0][nh],
            "bigq": np.ascontiguousarray(bigq.astype(np_w)),
        })
    return in_maps


def _assemble(results):
    """Per-core [128 (mi,p), 512 (t',b)] partials -> n-half sums -> CRT."""
    yc = np.empty((B, 16 * L), dtype=np.float32)
    ys = np.empty((B, 16 * L), dtype=np.float32)
    for k in range(0, NCORES, 2):
        o0 = np.asarray(results[k]["out"]).astype(np.float32)
        o1 = np.asarray(results[k + 1]["out"]).astype(np.float32)
        o = o0 + o1                                  # add n-half partials
        slab = (
            o.reshape(128, NT, B)[:, ::-1, :].transpose(2, 1, 0)
            .reshape(B, 16 * MH)
        )
        dst = ys if k >= 4 else yc
        mh = ((k % 4) >> 1) & 1
        dst[:, 16 * MH * mh: 16 * MH * (mh + 1)] = slab
    y = np.empty((B, NB * 16), dtype=np.float32)
    y[:, : 16 * L] = 0.5 * (yc + ys)
    y[:, 16 * L:] = 0.5 * (yc - ys)
    return y


def kernel(x, blocks):
    global _last_results
    from concourse.bass_utils import run_bass_kernel_spmd

    nc = _get_program(DTYPE)
    in_maps = _prep_inputs(x, blocks, DTYPE)
    res = run_bass_kernel_spmd(nc, in_maps, list(range(NCORES)))
    _last_results = res
    return _assemble(res.results)


# revision 23
# speedup vs baseline: 1.0483x; 1.0483x over previous
# Block-circulant linear kernel for Trainium2 (Bass/Tile), 8-core SPMD.
#
# y[b, 16m+p] = sum_{n,q} blocks[(m-n)%512, p, q] * x[b, 16n+q]
#
# CRT split (exact): z^512-1 = (z^256-1)(z^256+1). With
#   Xc[n] = x[n] + x[n+256],  Xs[n] = x[n] - x[n+256]   (block index n)
#   Bc[s] = B[s] + B[s+256],  Bs[s] = B[s] - B[s+256]
# the problem becomes a 256-cyclic conv (Bc*Xc) and a 256-negacyclic conv
# (Bs*Xs); y[m] = (Yc[m]+Ys[m])/2, y[m+256] = (Yc[m]-Ys[m])/2. The fold /
# unfold are O(input) host-side adds; the device still performs the full
# contraction at half the MACs and ~0.45x the DMA bytes of the direct form.
#
# Sharding (2x2 per subproblem): each core computes a PARTIAL result for
# 128 output block-rows (m-half) over 128 contraction blocks (n-half);
# the host adds the two n-half partials. This doubles the PSUM tile count
# to 16 (full [128,512] PSUM bank, N=512 matmuls) while shrinking the
# BIGQ window to exactly L block-columns:
#   BIGQ[(ni,q), u*16+p] = sgn * Bh[(m0-n0 + u-ni - 128) % 256, p, q]
# All (m_tile t, n_chunk c) pairs with the same diagonal d = t - c share
# one stationary 128x128 tile: 31 accumulating matmuls per core.
#
# The xt layout is reversed (c' = 15 - c) and the psum t axis flipped
# (t' = 15 - t) so both streams are consumed in DMA arrival order.
import numpy as np

B = 32
NB = 512           # number of 16x16 blocks in the original problem
L = 256            # folded subproblem length (cyclic / negacyclic)
NCORES = 8
MH = 128           # output block-rows per core (m-half)
NH = 128           # contraction blocks per core (n-half)
NCH_C = NH // 8    # 16 n-chunks of 128 contraction rows
NT = MH // 8       # 16 psum tiles of 8 block-rows x 32 batch
ND = NCH_C + NT - 1  # 31 diagonal offsets d in [-15, 15]
W = 8 * ND + 8     # 256 BIGQ window width (in u units of 16 columns)
OFF = 8 * NCH_C    # window index offset: j = (m-n) + OFF
XT_COLS = NCH_C * B    # 512
OUT_COLS = NT * B      # 512

DTYPE = "float16"
NWARM = 3   # logical fp32 warm-up matmuls (2 instrs each, clock-boost trigger)
EARLY_COPY = True

_cached = {}
_last_results = None  # BassKernelResults of the most recent run (for profiling)


def _np_dtype(name):
    if name == "bfloat16":
        import ml_dtypes

        return ml_dtypes.bfloat16
    if name == "float16":
        return np.float16
    return np.float32


def _split_dt(dt_name):
    if ":" in dt_name:
        wt, mv = dt_name.split(":")
        return wt, mv
    return dt_name, dt_name


def _build_program(dt_name):
    import concourse.bacc as bacc
    import concourse.mybir as mybir
    import concourse.tile as tile

    wt_name, mv_name = _split_dt(dt_name)
    wdt = getattr(mybir.dt, wt_name)
    mdt = getattr(mybir.dt, mv_name)
    f32 = mybir.dt.float32

    # bigq chunks (cols). Chunk-completion semaphores fire ~1.2us after the
    # last byte (HBM write-receipt) and wait on the straggler of 16 queues,
    # so first/last chunks are small. The DRAM layout is CHUNK-MAJOR
    # ([128*NCH, csz] rows = chunk*128+partition) so each chunk is a fully
    # contiguous 128-256KB HBM read instead of 8KB-strided rows.
    bq_cuts = [0, 512, 1536, 2560, 3584, 4096]

    nc = bacc.Bacc("TRN2", target_bir_lowering=False, debug=False, num_devices=NCORES)
    xt_d = nc.declare_dram_parameter("xt", [128, XT_COLS], mdt, isOutput=False)
    bq_ds = [
        nc.declare_dram_parameter(f"bigq{ci}", [128, hi - lo], wdt, isOutput=False)
        for ci, (lo, hi) in enumerate(zip(bq_cuts[:-1], bq_cuts[1:]))
    ]
    out_d = nc.declare_dram_parameter("out", [128, OUT_COLS], mdt, isOutput=True)

    with tile.TileContext(nc) as tc:
        with (
            tc.tile_pool(name="data", bufs=1) as data_pool,
            tc.tile_pool(name="psum", bufs=1, space="PSUM") as psum_pool,
        ):
            xt = data_pool.tile([128, XT_COLS], mdt)
            bq = data_pool.tile([128, W * 16], wdt)
            out_sb = data_pool.tile([128, OUT_COLS], mdt)
            warm_sb = data_pool.tile([128, 256], f32)
            acc = psum_pool.tile([128, OUT_COLS], f32)
            warm_ps = psum_pool.tile([128, 256], f32)

            # two HWDGE rings (sync=SP, scalar=ACT); consumption order.
            eng = [nc.scalar, nc.sync, nc.scalar, nc.sync, nc.scalar]
            eng[0].dma_start(bq[:, bq_cuts[0]:bq_cuts[1]], bq_ds[0][:])
            nc.sync.dma_start(xt[:], xt_d[:])
            for ci in range(1, len(bq_ds)):
                eng[ci].dma_start(bq[:, bq_cuts[ci]:bq_cuts[ci + 1]],
                                  bq_ds[ci][:])

            # PE warm-up nudge while DMA streams in (the HAM clock gate is
            # usually already warm from the preceding NEFF execution).
            nwarm = NWARM if mv_name in ("float16", "bfloat16") else 0
            if nwarm:
                nc.gpsimd.memset(warm_sb[:], 0.0)
            for wi in range(nwarm):
                nc.tensor.matmul(
                    warm_ps[:], warm_sb[:, 0:128], warm_sb[:],
                    start=(wi == 0), stop=(wi == nwarm - 1),
                )

            # d = t - c diagonal; stationary tile = BIGQ cols [16u0, 16u0+128)
            for i in range(ND):
                d = i - (NCH_C - 1)
                u0 = 8 * i + 8
                t_lo = max(0, d)
                t_hi = min(NT - 1, NCH_C - 1 + d)
                nt = t_hi - t_lo + 1
                tp_lo = NT - 1 - t_hi            # flipped psum tile index
                cp_lo = NCH_C - 1 + d - t_hi     # reversed xt chunk index
                nc.tensor.matmul(
                    acc[:, 32 * tp_lo: 32 * (tp_lo + nt)],
                    bq[:, 16 * u0: 16 * u0 + 128],
                    xt[:, 32 * cp_lo: 32 * (cp_lo + nt)],
                    start=(i == 0),   # clears the whole PSUM bank
                    stop=(i == ND - 1),
                )
                if EARLY_COPY and i == ND - 8:
                    # psum tiles t=0..8 (cols 256:512) got their last
                    # accumulation by i = NCH_C-1+t <= ND-8; cast them out
                    # while the remaining diagonals accumulate cols 0:256.
                    nc.scalar.copy(out_sb[:, 256:512], acc[:, 256:512])
                    nc.scalar.dma_start(out_d[:, 256:512], out_sb[:, 256:512])

            if EARLY_COPY:
                nc.vector.tensor_copy(out_sb[:, 0:256], acc[:, 0:256])
                nc.sync.dma_start(out_d[:, 0:256], out_sb[:, 0:256])
            else:
                nc.vector.tensor_copy(out_sb[:], acc[:])
                nc.sync.dma_start(out_d[:], out_sb[:])
    nc.compile()
    return nc


def _get_program(dt_name):
    key = (dt_name, NWARM, EARLY_COPY)
    if key not in _cached:
        _cached[key] = _build_program(dt_name)
    return _cached[key]


def _xt_layout(xh):
    """[32, 16*NH] n-half -> [128, XT_COLS]: xt[(ni*16+q), c'*32+b] with
    c' = NCH_C-1-c reversed chunk order."""
    xt = (
        xh.T.reshape(NCH_C, 128, B).transpose(1, 0, 2)[:, ::-1, :]
        .reshape(128, XT_COLS)
    )
    return np.ascontiguousarray(xt)


def _prep_inputs(x, blocks, dt_name):
    """Host-side fold + layout prep (numpy ops on the small inputs)."""
    x = np.ascontiguousarray(np.asarray(x), dtype=np.float32)
    blocks = np.ascontiguousarray(np.asarray(blocks), dtype=np.float32)
    wt_name, mv_name = _split_dt(dt_name)
    np_w, np_m = _np_dtype(wt_name), _np_dtype(mv_name)

    xc = x[:, : 16 * L] + x[:, 16 * L:]
    xs = x[:, : 16 * L] - x[:, 16 * L:]
    bc = blocks[:L] + blocks[L:]
    bs = blocks[:L] - blocks[L:]

    # xt per n-half: [cyc/neg][n-half]
    xts = [[_xt_layout(xh[:, 16 * NH * nh: 16 * NH * (nh + 1)]).astype(np_m)
            for nh in range(2)] for xh in (xc, xs)]

    u = np.arange(W)
    ni = np.arange(8)
    base = u[None, :] - ni[:, None] - OFF            # [8, W]
    in_maps = []
    for k in range(NCORES):
        neg = k >= 4
        mh, nh = ((k % 4) >> 1) & 1, k % 2
        s = (mh - nh) * NH + base                    # true (m-n) difference
        idx = s % L
        bh = bs if neg else bc
        bigq = bh[idx]                               # [8, W, p, q]
        if neg:
            sgn = (1.0 - 2.0 * ((s // L) % 2)).astype(np.float32)
            bigq = bigq * sgn[:, :, None, None]
        bigq = bigq.transpose(0, 3, 1, 2).reshape(128, W * 16)  # [(ni,q),(u,p)]
        bigq = bigq.astype(np_w)
        m = {"xt": xts[1 if neg else 0][nh]}
        cuts = [0, 512, 1536, 2560, 3584, 4096]
        for ci, (lo, hi) in enumerate(zip(cuts[:-1], cuts[1:])):
            m[f"bigq{ci}"] = np.ascontiguousarray(bigq[:, lo:hi])
        in_maps.append(m)
    return in_maps


def _assemble(results):
    """Per-core [128 (mi,p), 512 (t',b)] partials -> n-half sums -> CRT."""
    yc = np.empty((B, 16 * L), dtype=np.float32)
    ys = np.empty((B, 16 * L), dtype=np.float32)
    for k in range(0, NCORES, 2):
        o0 = np.asarray(results[k]["out"]).astype(np.float32)
        o1 = np.asarray(results[k + 1]["out"]).astype(np.float32)
        o = o0 + o1                                  # add n-half partials
        slab = (
            o.reshape(128, NT, B)[:, ::-1, :].transpose(2, 1, 0)
            .reshape(B, 16 * MH)
        )
        dst = ys if k >= 4 else yc
        mh = ((k % 4) >> 1) & 1
        dst[:, 16 * MH * mh: 16 * MH * (mh + 1)] = slab
    y = np.empty((B, NB * 16), dtype=np.float32)
    y[:, : 16 * L] = 0.5 * (yc + ys)
    y[:, 16 * L:] = 0.5 * (yc - ys)
    return y


def kernel(x, blocks):
    global _last_results
    from concourse.bass_utils import run_bass_kernel_spmd

    nc = _get_program(DTYPE)
    in_maps = _prep_inputs(x, blocks, DTYPE)
    res = run_bass_kernel_spmd(nc, in_maps, list(range(NCORES)))
    _last_results = res
    return _assemble(res.results)
